# revision 13
# baseline (speedup 1.0000x reference)
"""CrossAttentionBlock kernel for Trainium2 (Bass/Tile), 8-core data-parallel.

Strategy:
  - One batch element per NeuronCore (B=8 -> 8 cores), no collectives.
  - All activations kept feature-major ("transposed", [feature, token]) on
    device so every matmul contraction lands on the partition axis.
    Host pre-transposes query/key_value per core and pre-tiles all weight
    matrices into [m_tile, p, k_tile, col] blocks so every DMA is contiguous.
  - Matmuls run in float32r (TF32-like, 1 cycle/row) with fp32 PSUM accum.
  - LayerNorm reductions (over features = partitions) use ones-matmuls on the
    PE with M=128 so the stats come out pre-replicated across partitions.
  - Softmax: scores computed [key, query]-major; padding mask and 1/sqrt(d)
    scale fold into the Exp activation (bias/scale); the softmax denominator
    comes free as an extra ones-column in the attn@V matmul; no max-
    subtraction is needed (scores are small by construction).
  - attn_weights output (mean over heads of softmax) uses a second scores
    pass in [query, key] orientation where exp(s/8 - ln(16*l)) normalizes
    and averages for free; head-accumulation runs on GPSIMD to keep the
    vector engine off the critical path.
"""

import time

import numpy as np

import concourse.bass as bass
import concourse.tile as tile
from concourse import bacc, mybir
from concourse.bass_utils import run_bass_kernel_spmd

AF = mybir.ActivationFunctionType
ALU = mybir.AluOpType

f32 = mybir.dt.float32
f32r = mybir.dt.float32r

P = 128
D = 1024
H = 16
HD = 64
FF = 4096
B = 8
NQ = 512
NKV = 1024
KT = D // P  # 8 k-tiles over D
FT = FF // P  # 32 tiles over FF
MASK_NEG = -50.0
EPS = 1e-5

LAST_RESULTS = None


def _ln_partition_major(nc, work, psum_st, eps_col, x_sb, out_sb, free_len,
                        ones_sb, g_pm, b_pm):
    """LayerNorm over the partition (feature) axis of x_sb [P, KT, free_len].

    Stats are computed with ones-matmuls (M=128 -> replicated across
    partitions).  g_pm/b_pm are [P, KT] per-partition gamma/beta columns.
    out_sb may alias x_sb (in-place).
    """
    n_chunks = free_len // 512
    for c in range(n_chunks):
        cs = slice(c * 512, (c + 1) * 512)
        ps_mu = psum_st.tile([P, 512], f32, tag="st", bufs=2, name="ps_mu")
        ps_sq = psum_st.tile([P, 512], f32, tag="st", bufs=2, name="ps_sq")
        sq_tiles = []
        for kt in range(KT):
            sq = work.tile([P, 512], f32r, tag="ln_sq", bufs=3, name="ln_sq")
            nc.vector.tensor_mul(sq[:], x_sb[:, kt, cs], x_sb[:, kt, cs])
            sq_tiles.append(sq)
            nc.tensor.matmul(
                ps_sq[:], ones_sb[:], sq[:], start=kt == 0, stop=kt == KT - 1
            )
        for kt in range(KT):
            nc.tensor.matmul(
                ps_mu[:], ones_sb[:], x_sb[:, kt, cs], start=kt == 0, stop=kt == KT - 1
            )
        mu = work.tile([P, 512], f32, tag="ln_mu", bufs=1, name="ln_mu")
        nc.vector.tensor_scalar_mul(mu[:], ps_mu[:], 1.0 / D)
        musq = work.tile([P, 512], f32, tag="ln_musq", bufs=1, name="ln_musq")
        nc.vector.tensor_mul(musq[:], mu[:], mu[:])
        var = work.tile([P, 512], f32, tag="ln_var", bufs=1, name="ln_var")
        nc.vector.scalar_tensor_tensor(
            out=var[:],
            in0=ps_sq[:],
            scalar=1.0 / D,
            in1=musq[:],
            op0=ALU.mult,
            op1=ALU.subtract,
        )
        std = work.tile([P, 512], f32, tag="ln_std", bufs=1, name="ln_std")
        nc.scalar.activation(std[:], var[:], AF.Sqrt, bias=eps_col[:])
        rstd = work.tile([P, 512], f32, tag="ln_rstd", bufs=1, name="ln_rstd")
        nc.vector.reciprocal(rstd[:], std[:])
        for kt in range(KT):
            xc = work.tile([P, 512], f32, tag="ln_xc", bufs=3, name="ln_xc")
            nc.vector.tensor_sub(xc[:], x_sb[:, kt, cs], mu[:])
            # ((x - mu) * gamma) * rstd
            nc.vector.scalar_tensor_tensor(
                out=xc[:],
                in0=xc[:],
                scalar=g_pm[:, kt : kt + 1],
                in1=rstd[:],
                op0=ALU.mult,
                op1=ALU.mult,
            )
            nc.vector.tensor_scalar_add(
                out=out_sb[:, kt, cs], in0=xc[:], scalar1=b_pm[:, kt : kt + 1]
            )


def build_nc():
    nc = bacc.Bacc("TRN2", target_bir_lowering=False, debug=False)

    # ---- DRAM I/O ----
    q_t = nc.dram_tensor("query_t", [D, NQ], f32r, kind="ExternalInput")
    kv_t = nc.dram_tensor("kv_t", [D, NKV], f32r, kind="ExternalInput")
    maskb_d = nc.dram_tensor("maskbias_pm", [P, KT], f32, kind="ExternalInput")
    maskbit_d = nc.dram_tensor("maskbit_rep", [P, NKV], f32, kind="ExternalInput")
    w_qk_d = nc.dram_tensor("w_qk", [16, P, KT, P], f32r, kind="ExternalInput")
    w_v_d = nc.dram_tensor("w_v", [P, KT, D], f32r, kind="ExternalInput")
    ipb_d = nc.dram_tensor("ipb_pm", [P, 16], f32, kind="ExternalInput")
    bv_d = nc.dram_tensor("bv_rep", [P, D], f32, kind="ExternalInput")
    w_out_d = nc.dram_tensor("w_out", [KT, P, KT, P], f32r, kind="ExternalInput")
    outb_d = nc.dram_tensor("outb_pm", [P, KT], f32, kind="ExternalInput")
    gb_d = nc.dram_tensor("gb_pm", [P, 6, KT], f32, kind="ExternalInput")
    w_ff1_d = nc.dram_tensor("w_ff1", [FT, P, KT, P], f32r, kind="ExternalInput")
    ff1b_d = nc.dram_tensor("ff1b_pm", [P, FT], f32, kind="ExternalInput")
    w_ff2_d = nc.dram_tensor("w_ff2", [KT, P, FT, P], f32r, kind="ExternalInput")
    ff2b_d = nc.dram_tensor("ff2b_pm", [P, KT], f32, kind="ExternalInput")
    ones_d = nc.dram_tensor("ones_in", [P, P], f32r, kind="ExternalInput")

    x_t_out = nc.dram_tensor("x_t_out", [D, NQ], f32, kind="ExternalOutput")
    attn_out = nc.dram_tensor("attn_out", [NQ, NKV], f32, kind="ExternalOutput")

    with tile.TileContext(nc) as tc:
        # ---------- long-lived pools ----------
        # LEFT stack: const, p_x, p_qorig, p_ctx, p_qkT, p_v, p_att (LIFO)
        # RIGHT stack: p_kv, p_win, p_qln, work_in | p_wv | p_wmid, work_out
        const = tc.alloc_tile_pool(name="const", bufs=1, side="left")
        psum_mm = tc.alloc_tile_pool(name="psum_mm", bufs=6, space="PSUM")
        psum_st = tc.alloc_tile_pool(name="psum_st", bufs=1, space="PSUM")

        ones_sb = const.tile([P, P], f32r, tag="ones", name="ones_sb")
        nc.sync.dma_start(ones_sb[:], ones_d[:])
        eps_col = const.tile([P, 1], f32, tag="eps", name="eps_col")
        nc.vector.memset(eps_col[:], EPS)
        zero_col = const.tile([P, 1], f32, tag="zero", name="zero_col")
        nc.vector.memset(zero_col[:], 0.0)
        maskb = const.tile([P, KT], f32, tag="maskb", name="maskb")
        nc.sync.dma_start(maskb[:], maskb_d[:])
        maskbit = const.tile([P, NKV], f32, tag="maskbit", name="maskbit")
        nc.sync.dma_start(maskbit[:], maskbit_d[:])
        ipb = const.tile([P, 16], f32, tag="ipb", name="ipb")
        nc.sync.dma_start(ipb[:], ipb_d[:])
        bv = const.tile([P, D], f32, tag="bv", name="bv")
        nc.sync.dma_start(bv[:], bv_d[:])
        outb = const.tile([P, KT], f32, tag="outb", name="outb")
        nc.sync.dma_start(outb[:], outb_d[:])
        gb = const.tile([P, 6, KT], f32, tag="gb", name="gb")
        nc.sync.dma_start(gb[:], gb_d[:])
        ff1b = const.tile([P, FT], f32, tag="ff1b", name="ff1b")
        nc.sync.dma_start(ff1b[:], ff1b_d[:])
        ff2b = const.tile([P, KT], f32, tag="ff2b", name="ff2b")
        nc.sync.dma_start(ff2b[:], ff2b_d[:])
        lnl_t = const.tile([P, H, 4], f32, tag="lnl_t", name="lnl_t")
        lnl_dram = nc.dram_tensor("lnl_scratch", [H, NQ], f32)

        # ---------- phase pools ----------
        p_x = tc.alloc_tile_pool(name="p_x", bufs=1, side="left")
        p_qorig = tc.alloc_tile_pool(name="p_qorig", bufs=1, side="left")
        p_ctx = tc.alloc_tile_pool(name="p_ctx", bufs=1, side="left")
        p_qkT = tc.alloc_tile_pool(name="p_qkT", bufs=1, side="left")
        p_kv = tc.alloc_tile_pool(name="p_kv", bufs=1, side="right")
        p_win = tc.alloc_tile_pool(name="p_win", bufs=3, side="right")
        p_qln = tc.alloc_tile_pool(name="p_qln", bufs=1, side="right")
        work_in = tc.alloc_tile_pool(name="work_in", bufs=1, side="right")

        # ---- load activations (feature-major) ----
        q_orig = p_qorig.tile([P, KT, NQ], f32r, tag="q_orig", name="q_orig")
        for t in range(KT):
            nc.sync.dma_start(q_orig[:, t, :], q_t[t * P : (t + 1) * P, :])
        kv_sb = p_kv.tile([P, KT, NKV], f32r, tag="kv", name="kv_sb")
        for t in range(KT):
            nc.sync.dma_start(kv_sb[:, t, :], kv_t[t * P : (t + 1) * P, :])

        # ---- input layernorms (kv in-place) ----
        qln = p_qln.tile([P, KT, NQ], f32r, tag="qln", name="qln")
        _ln_partition_major(
            nc, work_in, psum_st, eps_col, q_orig, qln, NQ, ones_sb,
            gb[:, 0, :], gb[:, 1, :],
        )
        _ln_partition_major(
            nc, work_in, psum_st, eps_col, kv_sb, kv_sb, NKV, ones_sb,
            gb[:, 2, :], gb[:, 3, :],
        )

        # ---- in-projection: q.T, k.T (feature-major) ----
        qT = p_qkT.tile([P, KT, NQ], f32r, tag="qT", name="qT")
        kT = p_qkT.tile([P, KT, NKV], f32r, tag="kT", name="kT")
        for m in range(16):
            wt = p_win.tile([P, KT, P], f32r, tag="w", name="w_in")
            nc.sync.dma_start(wt[:], w_qk_d[m])
            if m < 8:  # q: one 512-wide chunk
                ps = psum_mm.tile([P, 512], f32, tag="mm", name="ps_q")
                for kt in range(KT):
                    nc.tensor.matmul(
                        ps[:], wt[:, kt, :], qln[:, kt, :],
                        start=kt == 0, stop=kt == KT - 1,
                    )
                nc.scalar.activation(
                    qT[:, m, :], ps[:], AF.Identity, bias=ipb[:, m : m + 1]
                )
            else:  # k: two 512-wide chunks
                for c in range(2):
                    cs = slice(c * 512, (c + 1) * 512)
                    ps = psum_mm.tile([P, 512], f32, tag="mm", name="ps_k")
                    for kt in range(KT):
                        nc.tensor.matmul(
                            ps[:], wt[:, kt, :], kv_sb[:, kt, cs],
                            start=kt == 0, stop=kt == KT - 1,
                        )
                    nc.scalar.activation(
                        kT[:, m - 8, cs], ps[:], AF.Identity,
                        bias=ipb[:, m : m + 1],
                    )
        work_in.release()
        p_qln.release()
        p_win.release()

        # ---- in-projection: v (token-major, [v | one] per head) ----
        p_v = tc.alloc_tile_pool(name="p_v", bufs=1, side="left")
        p_wv = tc.alloc_tile_pool(name="p_wv", bufs=1, side="right")
        v_sb = p_v.tile([P, KT, H, HD + 1], f32r, tag="v", name="v_sb")
        for tt in range(KT):
            nc.vector.tensor_copy(v_sb[:, tt, :, HD : HD + 1], ones_sb[:, 0:H, None])
        for c in range(2):
            wv_c = p_wv.tile([P, KT, 512], f32r, tag="wv", name="wv_c")
            nc.sync.dma_start(wv_c[:], w_v_d[:, :, c * 512 : (c + 1) * 512])
            for tt in range(KT):
                ps = psum_mm.tile([P, 512], f32, tag="mm", name="ps_v")
                for kt in range(KT):
                    nc.tensor.matmul(
                        ps[:],
                        kv_sb[:, kt, tt * P : (tt + 1) * P],
                        wv_c[:, kt, :],
                        start=kt == 0,
                        stop=kt == KT - 1,
                    )
                for j in range(8):
                    h = 8 * c + j
                    nc.vector.tensor_add(
                        v_sb[:, tt, h, 0:HD],
                        ps[:, 64 * j : 64 * j + 64],
                        bv[:, c * 512 + 64 * j : c * 512 + 64 * j + 64],
                    )
        p_wv.release()
        p_kv.release()

        # ---- attention ----
        p_att = tc.alloc_tile_pool(name="p_att", bufs=1, side="left")
        ctx_sb = p_ctx.tile([P, KT, NQ], f32r, tag="ctx", name="ctx_sb")
        attn_acc = p_att.tile([P, 4, NKV], f32, tag="attn_acc", name="attn_acc")
        for h in range(H):
            hb = (h % 2) * 64
            ht = h // 2
            hs = slice(hb, hb + 64)
            # scores pass 1: s.T [key, query]; exp with mask+scale folded
            p_sb = p_att.tile([P, KT, NQ], f32r, tag="p", bufs=2, name="p_sb")
            for tkt in range(KT):
                ps = psum_mm.tile([P, 512], f32, tag="mm", name="ps_s1")
                nc.tensor.matmul(
                    ps[:],
                    kT[hs, ht, tkt * P : (tkt + 1) * P],
                    qT[hs, ht, :],
                    start=True,
                    stop=True,
                )
                nc.scalar.activation(
                    p_sb[:, tkt, :], ps[:], AF.Exp,
                    bias=maskb[:, tkt : tkt + 1], scale=0.125,
                )
            # ctx.T + softmax denominator (ones column)
            ctx_ps = psum_mm.tile([P, 512], f32, tag="mm", name="ps_ctx")
            for tt in range(KT):
                nc.tensor.matmul(
                    ctx_ps[0:65, :],
                    v_sb[:, tt, h, :],
                    p_sb[:, tt, :],
                    start=tt == 0,
                    stop=tt == KT - 1,
                )
            # broadcast l across partitions with a K=1 ones-matmul, then
            # reciprocal -> r_rep [P, 512]
            l_row = p_att.tile([P, 512], f32r, tag="lrow", bufs=2, name="l_row")
            nc.vector.tensor_copy(l_row[64:65, :], ctx_ps[64:65, :])
            l_rep = psum_mm.tile([P, 512], f32, tag="mm", name="l_rep")
            nc.tensor.matmul(
                l_rep[:], ones_sb[64:65, :], l_row[64:65, :], start=True, stop=True
            )
            r_rep = p_att.tile([P, 512], f32, tag="rrep", bufs=2, name="r_rep")
            nc.vector.reciprocal(r_rep[:], l_rep[:])
            # -ln(16*l) = Ln(r/16); bounce through DRAM to transpose the
            # per-query vector into per-partition bias columns
            nsc = p_att.tile([P, 512], f32, tag="nsc", bufs=2, name="nsc")
            nc.scalar.activation(
                nsc[64:65, :], r_rep[64:65, :], AF.Ln,
                bias=zero_col[64:65, :], scale=1.0 / 16.0,
            )
            nc.sync.dma_start(lnl_dram[h], nsc[64:65, :])
            nc.sync.dma_start(
                lnl_t[:, h, :], lnl_dram[h].rearrange("(t p) -> p t", p=P)
            )
            # normalized ctx into feature-major ctx_sb
            if h % 2 == 0:
                nc.vector.tensor_mul(
                    ctx_sb[0:64, ht, :], ctx_ps[0:64, :], r_rep[0:64, :]
                )
            else:
                ctmp = p_att.tile([64, 512], f32r, tag="ctmp", bufs=1, name="ctmp")
                nc.vector.tensor_mul(ctmp[:], ctx_ps[0:64, :], r_rep[0:64, :])
                nc.sync.dma_start(ctx_sb[64:128, ht, :], ctmp[:])
            # scores pass 2: [query, key]; exp(s/8 - ln(16 l)) accumulated
            for tqt in range(4):
                for c in range(2):
                    cs = slice(c * 512, (c + 1) * 512)
                    ps2 = psum_mm.tile([P, 512], f32, tag="mm", name="ps_s2")
                    nc.tensor.matmul(
                        ps2[:],
                        qT[hs, ht, tqt * P : (tqt + 1) * P],
                        kT[hs, ht, cs],
                        start=True,
                        stop=True,
                    )
                    pn = p_att.tile([P, 512], f32, tag="pn", bufs=2, name="pn")
                    nc.scalar.activation(
                        pn[:], ps2[:], AF.Exp,
                        bias=lnl_t[:, h, tqt : tqt + 1], scale=0.125,
                    )
                    if h == 0:
                        nc.gpsimd.tensor_copy(attn_acc[:, tqt, cs], pn[:])
                    else:
                        nc.gpsimd.tensor_add(
                            attn_acc[:, tqt, cs], attn_acc[:, tqt, cs], pn[:]
                        )

        # mask masked keys back to exactly 0 and store attn output
        for tqt in range(4):
            nc.vector.tensor_mul(
                attn_acc[:, tqt, :], attn_acc[:, tqt, :], maskbit[:]
            )
            nc.sync.dma_start(
                attn_out[tqt * P : (tqt + 1) * P, :], attn_acc[:, tqt, :]
            )
        p_att.release()
        p_v.release()
        p_qkT.release()

        # ---- out-projection + residual ----
        p_wmid = tc.alloc_tile_pool(name="p_wmid", bufs=3, side="right")
        work_out = tc.alloc_tile_pool(name="work_out", bufs=1, side="right")
        x_sb = p_x.tile([P, KT, NQ], f32r, tag="x", name="x_sb")
        for m in range(KT):
            wt = p_wmid.tile([P, KT, P], f32r, tag="w", name="w_out_t")
            nc.sync.dma_start(wt[:], w_out_d[m])
            ps = psum_mm.tile([P, 512], f32, tag="mm", name="ps_o")
            for kt in range(KT):
                nc.tensor.matmul(
                    ps[:], wt[:, kt, :], ctx_sb[:, kt, :],
                    start=kt == 0, stop=kt == KT - 1,
                )
            # x = (attended + out_b) + query
            nc.vector.scalar_tensor_tensor(
                out=x_sb[:, m, :],
                in0=ps[:],
                scalar=outb[:, m : m + 1],
                in1=q_orig[:, m, :],
                op0=ALU.add,
                op1=ALU.add,
            )
        p_ctx.release()
        p_qorig.release()

        # ---- FFN ----
        p_ffn = tc.alloc_tile_pool(name="p_ffn", bufs=1, side="left")
        xln = p_ffn.tile([P, KT, NQ], f32r, tag="xln", name="xln")
        _ln_partition_major(
            nc, work_out, psum_st, eps_col, x_sb, xln, NQ, ones_sb,
            gb[:, 4, :], gb[:, 5, :],
        )
        h_sb = p_ffn.tile([P, FT, NQ], f32r, tag="h", name="h_sb")
        for m in range(FT):
            wt = p_wmid.tile([P, KT, P], f32r, tag="w", name="w_ff1_t")
            nc.sync.dma_start(wt[:], w_ff1_d[m])
            ps = psum_mm.tile([P, 512], f32, tag="mm", name="ps_f1")
            for kt in range(KT):
                nc.tensor.matmul(
                    ps[:], wt[:, kt, :], xln[:, kt, :],
                    start=kt == 0, stop=kt == KT - 1,
                )
            nc.scalar.activation(
                h_sb[:, m, :], ps[:], AF.Gelu, bias=ff1b[:, m : m + 1]
            )
        out_sb = p_ffn.tile([P, KT, NQ], f32, tag="out", name="out_sb")
        for m in range(KT):
            wt = p_wmid.tile([P, FT, P], f32r, tag="wff2", bufs=2, name="w_ff2_t")
            nc.sync.dma_start(wt[:], w_ff2_d[m])
            ps = psum_mm.tile([P, 512], f32, tag="mm", name="ps_f2")
            for kt in range(FT):
                nc.tensor.matmul(
                    ps[:], wt[:, kt, :], h_sb[:, kt, :],
                    start=kt == 0, stop=kt == FT - 1,
                )
            nc.vector.scalar_tensor_tensor(
                out=out_sb[:, m, :],
                in0=ps[:],
                scalar=ff2b[:, m : m + 1],
                in1=x_sb[:, m, :],
                op0=ALU.add,
                op1=ALU.add,
            )
            nc.sync.dma_start(x_t_out[m * P : (m + 1) * P, :], out_sb[:, m, :])

        p_ffn.release()
        p_x.release()
        work_out.release()
        p_wmid.release()
        const.release()
        psum_st.release()
        psum_mm.release()

    nc.compile()
    return nc


_NC_CACHE = None


def _get_nc():
    global _NC_CACHE
    if _NC_CACHE is None:
        _NC_CACHE = build_nc()
    return _NC_CACHE


def _prep_shared(in_proj_w, in_proj_b, out_w, out_b, nq_gamma, nq_beta, nkv_gamma,
                 nkv_beta, nff_gamma, nff_beta, ff1_w, ff1_b, ff2_w, ff2_b):
    def pm(v, nt):  # per-partition layout [P, nt]
        return np.ascontiguousarray(np.asarray(v, np.float32).reshape(nt, P).T)

    def wtiles(w_t, mt):  # [m, p, kt, c] tiled layout from [in, out] matrix
        kt = w_t.shape[0] // P
        return np.ascontiguousarray(w_t.reshape(kt, P, mt, P).transpose(2, 1, 0, 3))

    ipw_t = np.asarray(in_proj_w, np.float32).T  # (1024, 3072)
    return {
        "w_qk": wtiles(np.ascontiguousarray(ipw_t[:, : 2 * D]), 16),
        "w_v": np.ascontiguousarray(
            ipw_t[:, 2 * D :].reshape(KT, P, D).transpose(1, 0, 2)
        ),
        "ipb_pm": pm(np.asarray(in_proj_b, np.float32)[: 2 * D], 16),
        "bv_rep": np.ascontiguousarray(
            np.broadcast_to(np.asarray(in_proj_b, np.float32)[2 * D :], (P, D))
        ),
        "w_out": wtiles(np.asarray(out_w, np.float32).T, KT),
        "outb_pm": pm(out_b, KT),
        "gb_pm": np.ascontiguousarray(
            np.stack(
                [pm(v, KT) for v in
                 [nq_gamma, nq_beta, nkv_gamma, nkv_beta, nff_gamma, nff_beta]],
                axis=1,
            )
        ),
        "w_ff1": wtiles(np.asarray(ff1_w, np.float32).T, FT),
        "ff1b_pm": pm(ff1_b, FT),
        "w_ff2": wtiles(np.asarray(ff2_w, np.float32).T, KT),
        "ff2b_pm": pm(ff2_b, KT),
    }


def kernel(query, key_value, key_padding_mask, nq_gamma, nq_beta, nkv_gamma,
           nkv_beta, in_proj_w, in_proj_b, out_w, out_b, nff_gamma, nff_beta,
           ff1_w, ff1_b, ff2_w, ff2_b):
    global LAST_RESULTS
    query = np.asarray(query, np.float32)
    key_value = np.asarray(key_value, np.float32)
    mask = np.asarray(key_padding_mask)

    shared = _prep_shared(in_proj_w, in_proj_b, out_w, out_b, nq_gamma, nq_beta,
                          nkv_gamma, nkv_beta, nff_gamma, nff_beta, ff1_w,
                          ff1_b, ff2_w, ff2_b)

    in_maps = []
    for b in range(B):
        mb = np.where(mask[b], np.float32(MASK_NEG), np.float32(0.0)).astype(
            np.float32
        )
        mbit = np.where(mask[b], np.float32(0.0), np.float32(1.0)).astype(np.float32)
        m = dict(shared)
        m["query_t"] = np.ascontiguousarray(query[b].T)
        m["kv_t"] = np.ascontiguousarray(key_value[b].T)
        m["maskbias_pm"] = np.ascontiguousarray(mb.reshape(KT, P).T)
        m["ones_in"] = np.ones((P, P), np.float32)
        m["maskbit_rep"] = np.ascontiguousarray(np.broadcast_to(mbit, (P, NKV)))
        in_maps.append(m)

    nc = _get_nc()
    t0 = time.monotonic()
    res = run_bass_kernel_spmd(nc, in_maps, core_ids=list(range(B)))
    t1 = time.monotonic()
    LAST_RESULTS = {"res": res, "wall_s": t1 - t0}

    x = np.stack([res.results[b]["x_t_out"].T for b in range(B)])
    attn = np.stack([res.results[b]["attn_out"] for b in range(B)])
    return (np.ascontiguousarray(x), np.ascontiguousarray(attn))


# revision 18
# speedup vs baseline: 1.1248x; 1.1248x over previous
"""CrossAttentionBlock kernel for Trainium2 (Bass/Tile), 8-core data-parallel.

Strategy:
  - One batch element per NeuronCore (B=8 -> 8 cores), no collectives.
  - All activations kept feature-major ("transposed", [feature, token]) on
    device so every matmul contraction lands on the partition axis.
    Host pre-transposes query/key_value per core and pre-tiles all weight
    matrices into [m_tile, p, k_tile, col] blocks so every DMA is contiguous.
  - Matmuls run in float32r (TF32-like, 1 cycle/row) with fp32 PSUM accum.
  - LayerNorm reductions (over features = partitions) use ones-matmuls on the
    PE with M=128 so the stats come out pre-replicated across partitions.
  - Softmax: scores computed [key, query]-major; padding mask and 1/sqrt(d)
    scale fold into the Exp activation (bias/scale); the softmax denominator
    comes free as an extra ones-column in the attn@V matmul; no max-
    subtraction is needed (scores are small by construction).
  - attn_weights output (mean over heads of softmax) uses a second scores
    pass in [query, key] orientation where exp(s/8 - ln(16*l)) normalizes
    and averages for free; head-accumulation runs on GPSIMD to keep the
    vector engine off the critical path.
"""

import time

import ml_dtypes
import numpy as np

import concourse.bass as bass
import concourse.tile as tile
from concourse import bacc, mybir
from concourse.bass_utils import run_bass_kernel_spmd

AF = mybir.ActivationFunctionType
ALU = mybir.AluOpType

f32 = mybir.dt.float32
f32r = mybir.dt.float32r
bf16 = mybir.dt.bfloat16

P = 128
D = 1024
H = 16
HD = 64
FF = 4096
B = 8
NQ = 512
NKV = 1024
KT = D // P  # 8 k-tiles over D
FT = FF // P  # 32 tiles over FF
MASK_NEG = -50.0
EPS = 1e-5

LAST_RESULTS = None


def _ln_partition_major(nc, work, psum_st, eps_col, zero_col, x_sb, out_sb, free_len,
                        ones_sb, g_pm, b_pm):
    """LayerNorm over the partition (feature) axis of x_sb [P, KT, free_len].

    Stats are computed with ones-matmuls (M=128 -> replicated across
    partitions).  g_pm/b_pm are [P, KT] per-partition gamma/beta columns.
    out_sb may alias x_sb (in-place).
    """
    n_chunks = free_len // 512
    for c in range(n_chunks):
        cs = slice(c * 512, (c + 1) * 512)
        ps_mu = psum_st.tile([P, 512], f32, tag="mm", name="ps_mu")
        ps_sq = psum_st.tile([P, 512], f32, tag="mm", name="ps_sq")
        sq_tiles = []
        for kt in range(KT):
            sq = work.tile([P, 512], f32r, tag="ln_sq", bufs=4, name="ln_sq")
            if kt % 2 == 0:
                nc.vector.tensor_mul(sq[:], x_sb[:, kt, cs], x_sb[:, kt, cs])
            else:
                nc.scalar.activation(sq[:], x_sb[:, kt, cs], AF.Square,
                                     bias=zero_col[:], scale=1.0)
            sq_tiles.append(sq)
            nc.tensor.matmul(
                ps_sq[:], ones_sb[:], sq[:], start=kt == 0, stop=kt == KT - 1
            )
        for kt in range(KT):
            nc.tensor.matmul(
                ps_mu[:], ones_sb[:], x_sb[:, kt, cs], start=kt == 0, stop=kt == KT - 1
            )
        mu = work.tile([P, 512], f32, tag="ln_mu", bufs=1, name="ln_mu")
        nc.vector.tensor_scalar_mul(mu[:], ps_mu[:], 1.0 / D)
        musq = work.tile([P, 512], f32, tag="ln_musq", bufs=1, name="ln_musq")
        nc.vector.tensor_mul(musq[:], mu[:], mu[:])
        var = work.tile([P, 512], f32, tag="ln_var", bufs=1, name="ln_var")
        nc.vector.scalar_tensor_tensor(
            out=var[:],
            in0=ps_sq[:],
            scalar=1.0 / D,
            in1=musq[:],
            op0=ALU.mult,
            op1=ALU.subtract,
        )
        std = work.tile([P, 512], f32, tag="ln_std", bufs=1, name="ln_std")
        nc.scalar.activation(std[:], var[:], AF.Sqrt, bias=eps_col[:])
        rstd = work.tile([P, 512], f32, tag="ln_rstd", bufs=1, name="ln_rstd")
        nc.vector.reciprocal(rstd[:], std[:])
        for kt in range(KT):
            xc = work.tile([P, 512], f32, tag="ln_xc", bufs=4, name="ln_xc")
            nc.vector.tensor_sub(xc[:], x_sb[:, kt, cs], mu[:])
            nc.vector.tensor_mul(xc[:], xc[:], rstd[:])
            # gamma * xc + beta on the scalar engine
            nc.scalar.activation(
                out_sb[:, kt, cs], xc[:], AF.Identity,
                bias=b_pm[:, kt : kt + 1], scale=g_pm[:, kt : kt + 1],
            )


def build_nc():
    nc = bacc.Bacc("TRN2", target_bir_lowering=False, debug=False)

    # ---- DRAM I/O ----
    q_t = nc.dram_tensor("query_t", [D, NQ], f32r, kind="ExternalInput")
    kv_t = nc.dram_tensor("kv_t", [D, NKV], f32r, kind="ExternalInput")
    maskb_d = nc.dram_tensor("maskbias_pm", [P, KT], f32, kind="ExternalInput")
    maskbit_d = nc.dram_tensor("maskbit16_pm", [P, KT], f32, kind="ExternalInput")
    w_qk_d = nc.dram_tensor("w_qk", [16, P, KT, P], f32r, kind="ExternalInput")
    w_v_d = nc.dram_tensor("w_v", [P, KT, D], f32r, kind="ExternalInput")
    ipb_d = nc.dram_tensor("ipb_pm", [P, 16], f32, kind="ExternalInput")
    bv_d = nc.dram_tensor("bv_rep", [P, D], f32, kind="ExternalInput")
    w_out_d = nc.dram_tensor("w_out", [KT, P, KT, P], f32r, kind="ExternalInput")
    outb_d = nc.dram_tensor("outb_pm", [P, KT], f32, kind="ExternalInput")
    gb_d = nc.dram_tensor("gb_pm", [P, 6, KT], f32, kind="ExternalInput")
    w_ff1_d = nc.dram_tensor("w_ff1", [FT, P, KT, P], bf16, kind="ExternalInput")
    ff1b_d = nc.dram_tensor("ff1b_pm", [P, FT], f32, kind="ExternalInput")
    w_ff2_d = nc.dram_tensor("w_ff2", [KT, P, FT, P], bf16, kind="ExternalInput")
    ff2b_d = nc.dram_tensor("ff2b_pm", [P, KT], f32, kind="ExternalInput")
    ones_d = nc.dram_tensor("ones_in", [P, P], f32r, kind="ExternalInput")

    x_t_out = nc.dram_tensor("x_t_out", [D, NQ], f32, kind="ExternalOutput")
    attn_out = nc.dram_tensor("attn_t_out", [NKV, NQ], f32, kind="ExternalOutput")

    with tile.TileContext(nc) as tc:
        # ---------- long-lived pools ----------
        # LEFT stack: const, p_x, p_qorig, p_ctx, p_qkT, p_v, p_att (LIFO)
        # RIGHT stack: p_kv, p_win, p_qln, work_in | p_wv | p_wmid, work_out
        const = tc.alloc_tile_pool(name="const", bufs=1, side="left")
        psum_mm = tc.alloc_tile_pool(name="psum_mm", bufs=8, space="PSUM")

        ones_sb = const.tile([P, P], f32r, tag="ones", name="ones_sb")
        nc.sync.dma_start(ones_sb[:], ones_d[:])
        eps_col = const.tile([P, 1], f32, tag="eps", name="eps_col")
        nc.vector.memset(eps_col[:], EPS)
        zero_col = const.tile([P, 1], f32, tag="zero", name="zero_col")
        nc.vector.memset(zero_col[:], 0.0)
        maskb = const.tile([P, KT], f32, tag="maskb", name="maskb")
        nc.sync.dma_start(maskb[:], maskb_d[:])
        maskbit = const.tile([P, KT], f32, tag="maskbit", name="maskbit")
        nc.sync.dma_start(maskbit[:], maskbit_d[:])
        ipb = const.tile([P, 16], f32, tag="ipb", name="ipb")
        nc.sync.dma_start(ipb[:], ipb_d[:])
        bv = const.tile([P, D], f32, tag="bv", name="bv")
        nc.sync.dma_start(bv[:], bv_d[:])
        outb = const.tile([P, KT], f32, tag="outb", name="outb")
        nc.sync.dma_start(outb[:], outb_d[:])
        gb = const.tile([P, 6, KT], f32, tag="gb", name="gb")
        nc.sync.dma_start(gb[:], gb_d[:])
        ff1b = const.tile([P, FT], f32, tag="ff1b", name="ff1b")
        nc.sync.dma_start(ff1b[:], ff1b_d[:])
        ff2b = const.tile([P, KT], f32, tag="ff2b", name="ff2b")
        nc.sync.dma_start(ff2b[:], ff2b_d[:])

        # ---------- phase pools ----------
        p_x = tc.alloc_tile_pool(name="p_x", bufs=1, side="left")
        p_qorig = tc.alloc_tile_pool(name="p_qorig", bufs=1, side="left")
        p_ctx = tc.alloc_tile_pool(name="p_ctx", bufs=1, side="left")
        p_qkT = tc.alloc_tile_pool(name="p_qkT", bufs=1, side="left")
        p_kv = tc.alloc_tile_pool(name="p_kv", bufs=1, side="right")
        p_win = tc.alloc_tile_pool(name="p_win", bufs=3, side="right")
        p_qln = tc.alloc_tile_pool(name="p_qln", bufs=1, side="right")
        work_in = tc.alloc_tile_pool(name="work_in", bufs=1, side="right")

        # ---- load activations (feature-major) ----
        q_orig = p_qorig.tile([P, KT, NQ], f32r, tag="q_orig", name="q_orig")
        for t in range(KT):
            nc.sync.dma_start(q_orig[:, t, :], q_t[t * P : (t + 1) * P, :])
        kv_sb = p_kv.tile([P, KT, NKV], f32r, tag="kv", name="kv_sb")
        for t in range(KT):
            nc.sync.dma_start(kv_sb[:, t, :], kv_t[t * P : (t + 1) * P, :])

        # ---- input layernorms (kv in-place) ----
        qln = p_qln.tile([P, KT, NQ], f32r, tag="qln", name="qln")
        _ln_partition_major(
            nc, work_in, psum_mm, eps_col, zero_col, q_orig, qln, NQ, ones_sb,
            gb[:, 0, :], gb[:, 1, :],
        )
        _ln_partition_major(
            nc, work_in, psum_mm, eps_col, zero_col, kv_sb, kv_sb, NKV, ones_sb,
            gb[:, 2, :], gb[:, 3, :],
        )

        # ---- in-projection: q.T, k.T (feature-major) ----
        qT = p_qkT.tile([P, KT, NQ], f32r, tag="qT", name="qT")
        kT = p_qkT.tile([P, KT, NKV], f32r, tag="kT", name="kT")
        for m in range(16):
            wt = p_win.tile([P, KT, P], f32r, tag="w", name="w_in")
            nc.sync.dma_start(wt[:], w_qk_d[m])
            if m < 8:  # q: one 512-wide chunk
                ps = psum_mm.tile([P, 512], f32, tag="mm", name="ps_q")
                for kt in range(KT):
                    nc.tensor.matmul(
                        ps[:], wt[:, kt, :], qln[:, kt, :],
                        start=kt == 0, stop=kt == KT - 1,
                    )
                if m % 2 == 0:
                    nc.scalar.activation(
                        qT[:, m, :], ps[:], AF.Identity, bias=ipb[:, m : m + 1]
                    )
                else:
                    nc.vector.tensor_scalar_add(
                        out=qT[:, m, :], in0=ps[:], scalar1=ipb[:, m : m + 1]
                    )
            else:  # k: two 512-wide chunks
                for c in range(2):
                    cs = slice(c * 512, (c + 1) * 512)
                    ps = psum_mm.tile([P, 512], f32, tag="mm", name="ps_k")
                    for kt in range(KT):
                        nc.tensor.matmul(
                            ps[:], wt[:, kt, :], kv_sb[:, kt, cs],
                            start=kt == 0, stop=kt == KT - 1,
                        )
                    if m % 2 == 0:
                        nc.scalar.activation(
                            kT[:, m - 8, cs], ps[:], AF.Identity,
                            bias=ipb[:, m : m + 1],
                        )
                    else:
                        nc.vector.tensor_scalar_add(
                            out=kT[:, m - 8, cs], in0=ps[:],
                            scalar1=ipb[:, m : m + 1],
                        )
        work_in.release()
        p_qln.release()
        p_win.release()

        # ---- in-projection: v (token-major, [v | one] per head) ----
        p_v = tc.alloc_tile_pool(name="p_v", bufs=1, side="left")
        p_wv = tc.alloc_tile_pool(name="p_wv", bufs=1, side="right")
        v_sb = p_v.tile([P, KT, H, HD + 1], bf16, tag="v", name="v_sb")
        for tt in range(KT):
            nc.vector.tensor_copy(v_sb[:, tt, :, HD : HD + 1], ones_sb[:, 0:H, None])
        for c in range(2):
            wv_c = p_wv.tile([P, KT, 512], f32r, tag="wv", name="wv_c")
            nc.sync.dma_start(wv_c[:], w_v_d[:, :, c * 512 : (c + 1) * 512])
            for tt in range(KT):
                ps = psum_mm.tile([P, 512], f32, tag="mm", name="ps_v")
                for kt in range(KT):
                    nc.tensor.matmul(
                        ps[:],
                        kv_sb[:, kt, tt * P : (tt + 1) * P],
                        wv_c[:, kt, :],
                        start=kt == 0,
                        stop=kt == KT - 1,
                    )
                nc.vector.tensor_add(
                    v_sb[:, tt, 8 * c : 8 * c + 8, 0:HD],
                    ps[:].rearrange("p (j d) -> p j d", d=HD),
                    bv[:, c * 512 : (c + 1) * 512].rearrange(
                        "p (j d) -> p j d", d=HD
                    ),
                )
        p_wv.release()
        p_kv.release()

        # ---- attention ----
        p_att = tc.alloc_tile_pool(name="p_att", bufs=1, side="left")
        ctx_sb = p_ctx.tile([P, KT, NQ], f32r, tag="ctx", name="ctx_sb")
        attn_acc = p_att.tile([P, KT, NQ], bf16, tag="attn_acc", name="attn_acc")
        nc.vector.memset(attn_acc[:], 0.0)
        for h in range(H):
            hb = (h % 2) * 64
            ht = h // 2
            hs = slice(hb, hb + 64)
            # scores pass 1: s.T [key, query]; exp with mask+scale folded
            p_sb = p_att.tile([P, KT, NQ], bf16, tag="p", bufs=2, name="p_sb")
            for tkt in range(KT):
                ps = psum_mm.tile([P, 512], f32, tag="mm", name="ps_s1")
                nc.tensor.matmul(
                    ps[:],
                    kT[hs, ht, tkt * P : (tkt + 1) * P],
                    qT[hs, ht, :],
                    start=True,
                    stop=True,
                )
                nc.scalar.activation(
                    p_sb[:, tkt, :], ps[:], AF.Exp,
                    bias=maskb[:, tkt : tkt + 1], scale=0.125,
                )
            # ctx.T + softmax denominator (ones column)
            ctx_ps = psum_mm.tile([P, 512], f32, tag="mm", name="ps_ctx")
            for tt in range(KT):
                nc.tensor.matmul(
                    ctx_ps[0:65, :],
                    v_sb[:, tt, h, :],
                    p_sb[:, tt, :],
                    start=tt == 0,
                    stop=tt == KT - 1,
                )
            # broadcast l across partitions with a K=1 ones-matmul, then
            # reciprocal -> r_rep [P, 512]
            l_row = p_att.tile([P, 512], f32r, tag="lrow", bufs=2, name="l_row")
            nc.vector.tensor_copy(l_row[64:65, :], ctx_ps[64:65, :])
            l_rep = psum_mm.tile([P, 512], f32, tag="mm", name="l_rep")
            nc.tensor.matmul(
                l_rep[:], ones_sb[64:65, :], l_row[64:65, :], start=True, stop=True
            )
            r_rep = p_att.tile([P, 512], f32, tag="rrep", bufs=2, name="r_rep")
            nc.vector.reciprocal(r_rep[:], l_rep[:])
            # normalized ctx into feature-major ctx_sb
            if h % 2 == 0:
                nc.vector.tensor_mul(
                    ctx_sb[0:64, ht, :], ctx_ps[0:64, :], r_rep[0:64, :]
                )
            else:
                ctmp = p_att.tile([64, 512], f32r, tag="ctmp", bufs=1, name="ctmp")
                nc.vector.tensor_mul(ctmp[:], ctx_ps[0:64, :], r_rep[0:64, :])
                nc.sync.dma_start(ctx_sb[64:128, ht, :], ctmp[:])
            # attn accumulation in [key, query] orientation:
            # acc[tk, tq] += p[tk, tq] * r[tq]   (mean/mask applied at the end)
            # bf16 pairs hit the DVE 2x mode; adds split between Pool and DVE
            r_bf = p_att.tile([P, 512], bf16, tag="rbf", bufs=2, name="r_bf")
            nc.vector.tensor_copy(r_bf[:], r_rep[:])
            for tkp in range(KT // 2):
                pr = p_att.tile([P, 2, 512], bf16, tag="pr", bufs=3, name="pr")
                nc.vector.tensor_mul(
                    pr[:],
                    p_sb[:, 2 * tkp : 2 * tkp + 2, :],
                    r_bf[:, None, :].to_broadcast([P, 2, 512]),
                )
                eng = nc.gpsimd if tkp < 2 else nc.vector
                eng.tensor_add(
                    attn_acc[:, 2 * tkp : 2 * tkp + 2, :],
                    attn_acc[:, 2 * tkp : 2 * tkp + 2, :],
                    pr[:],
                )

        # mean over heads + zero out masked keys, then store (transposed)
        for tkt in range(KT):
            nc.vector.tensor_scalar_mul(
                out=attn_acc[:, tkt, :], in0=attn_acc[:, tkt, :],
                scalar1=maskbit[:, tkt : tkt + 1],
            )
            nc.gpsimd.dma_start(
                attn_out[tkt * P : (tkt + 1) * P, :], attn_acc[:, tkt, :]
            )
        p_att.release()
        p_v.release()
        p_qkT.release()

        # ---- out-projection + residual ----
        p_wmid = tc.alloc_tile_pool(name="p_wmid", bufs=3, side="right")
        work_out = tc.alloc_tile_pool(name="work_out", bufs=1, side="right")
        x_sb = p_x.tile([P, KT, NQ], f32r, tag="x", name="x_sb")
        for m in range(KT):
            wt = p_wmid.tile([P, KT, P], f32r, tag="w", name="w_out_t")
            nc.sync.dma_start(wt[:], w_out_d[m])
            ps = psum_mm.tile([P, 512], f32, tag="mm", name="ps_o")
            for kt in range(KT):
                nc.tensor.matmul(
                    ps[:], wt[:, kt, :], ctx_sb[:, kt, :],
                    start=kt == 0, stop=kt == KT - 1,
                )
            # x = (attended + out_b) + query
            nc.vector.scalar_tensor_tensor(
                out=x_sb[:, m, :],
                in0=ps[:],
                scalar=outb[:, m : m + 1],
                in1=q_orig[:, m, :],
                op0=ALU.add,
                op1=ALU.add,
            )
        p_ctx.release()
        p_qorig.release()

        # ---- FFN ----
        p_ffn = tc.alloc_tile_pool(name="p_ffn", bufs=1, side="left")
        xln = p_ffn.tile([P, KT, NQ], bf16, tag="xln", name="xln")
        _ln_partition_major(
            nc, work_out, psum_mm, eps_col, zero_col, x_sb, xln, NQ, ones_sb,
            gb[:, 4, :], gb[:, 5, :],
        )
        h_sb = p_ffn.tile([P, FT, NQ], bf16, tag="h", name="h_sb")
        for m in range(FT):
            wt = p_wmid.tile([P, KT, P], bf16, tag="wb", name="w_ff1_t")
            nc.sync.dma_start(wt[:], w_ff1_d[m])
            ps = psum_mm.tile([P, 512], f32, tag="mm", name="ps_f1")
            for kt in range(KT):
                nc.tensor.matmul(
                    ps[:], wt[:, kt, :], xln[:, kt, :],
                    start=kt == 0, stop=kt == KT - 1,
                )
            nc.scalar.activation(
                h_sb[:, m, :], ps[:], AF.Gelu, bias=ff1b[:, m : m + 1]
            )
        out_sb = p_ffn.tile([P, KT, NQ], f32, tag="out", name="out_sb")
        for m in range(KT):
            wt = p_wmid.tile([P, FT, P], bf16, tag="wff2", bufs=2, name="w_ff2_t")
            nc.sync.dma_start(wt[:], w_ff2_d[m])
            ps = psum_mm.tile([P, 512], f32, tag="mm", name="ps_f2")
            for kt in range(FT):
                nc.tensor.matmul(
                    ps[:], wt[:, kt, :], h_sb[:, kt, :],
                    start=kt == 0, stop=kt == FT - 1,
                )
            nc.vector.scalar_tensor_tensor(
                out=out_sb[:, m, :],
                in0=ps[:],
                scalar=ff2b[:, m : m + 1],
                in1=x_sb[:, m, :],
                op0=ALU.add,
                op1=ALU.add,
            )
            nc.sync.dma_start(x_t_out[m * P : (m + 1) * P, :], out_sb[:, m, :])

        p_ffn.release()
        p_x.release()
        work_out.release()
        p_wmid.release()
        const.release()
        psum_mm.release()

    nc.compile()
    return nc


_NC_CACHE = None


def _get_nc():
    global _NC_CACHE
    if _NC_CACHE is None:
        _NC_CACHE = build_nc()
    return _NC_CACHE


def _prep_shared(in_proj_w, in_proj_b, out_w, out_b, nq_gamma, nq_beta, nkv_gamma,
                 nkv_beta, nff_gamma, nff_beta, ff1_w, ff1_b, ff2_w, ff2_b):
    def pm(v, nt):  # per-partition layout [P, nt]
        return np.ascontiguousarray(np.asarray(v, np.float32).reshape(nt, P).T)

    def wtiles(w_t, mt):  # [m, p, kt, c] tiled layout from [in, out] matrix
        kt = w_t.shape[0] // P
        return np.ascontiguousarray(w_t.reshape(kt, P, mt, P).transpose(2, 1, 0, 3))

    ipw_t = np.asarray(in_proj_w, np.float32).T  # (1024, 3072)
    return {
        "w_qk": wtiles(np.ascontiguousarray(ipw_t[:, : 2 * D]), 16),
        "w_v": np.ascontiguousarray(
            ipw_t[:, 2 * D :].reshape(KT, P, D).transpose(1, 0, 2)
        ),
        "ipb_pm": pm(np.asarray(in_proj_b, np.float32)[: 2 * D], 16),
        "bv_rep": np.ascontiguousarray(
            np.broadcast_to(np.asarray(in_proj_b, np.float32)[2 * D :], (P, D))
        ),
        "w_out": wtiles(np.asarray(out_w, np.float32).T, KT),
        "outb_pm": pm(out_b, KT),
        "gb_pm": np.ascontiguousarray(
            np.stack(
                [pm(v, KT) for v in
                 [nq_gamma, nq_beta, nkv_gamma, nkv_beta, nff_gamma, nff_beta]],
                axis=1,
            )
        ),
        "w_ff1": wtiles(np.asarray(ff1_w, np.float32).T, FT).astype(
            ml_dtypes.bfloat16
        ),
        "ff1b_pm": pm(ff1_b, FT),
        "w_ff2": wtiles(np.asarray(ff2_w, np.float32).T, KT).astype(
            ml_dtypes.bfloat16
        ),
        "ff2b_pm": pm(ff2_b, KT),
    }


def kernel(query, key_value, key_padding_mask, nq_gamma, nq_beta, nkv_gamma,
           nkv_beta, in_proj_w, in_proj_b, out_w, out_b, nff_gamma, nff_beta,
           ff1_w, ff1_b, ff2_w, ff2_b):
    global LAST_RESULTS
    query = np.asarray(query, np.float32)
    key_value = np.asarray(key_value, np.float32)
    mask = np.asarray(key_padding_mask)

    shared = _prep_shared(in_proj_w, in_proj_b, out_w, out_b, nq_gamma, nq_beta,
                          nkv_gamma, nkv_beta, nff_gamma, nff_beta, ff1_w,
                          ff1_b, ff2_w, ff2_b)

    in_maps = []
    for b in range(B):
        mb = np.where(mask[b], np.float32(MASK_NEG), np.float32(0.0)).astype(
            np.float32
        )
        mbit = np.where(mask[b], np.float32(0.0), np.float32(1.0 / 16.0)).astype(
            np.float32
        )
        m = dict(shared)
        m["query_t"] = np.ascontiguousarray(query[b].T)
        m["kv_t"] = np.ascontiguousarray(key_value[b].T)
        m["maskbias_pm"] = np.ascontiguousarray(mb.reshape(KT, P).T)
        m["ones_in"] = np.ones((P, P), np.float32)
        m["maskbit16_pm"] = np.ascontiguousarray(mbit.reshape(KT, P).T)
        in_maps.append(m)

    nc = _get_nc()
    t0 = time.monotonic()
    res = run_bass_kernel_spmd(nc, in_maps, core_ids=list(range(B)))
    t1 = time.monotonic()
    LAST_RESULTS = {"res": res, "wall_s": t1 - t0}

    x = np.stack([res.results[b]["x_t_out"].T for b in range(B)])
    attn = np.stack([res.results[b]["attn_t_out"].T for b in range(B)])
    return (np.ascontiguousarray(x), np.ascontiguousarray(attn))


# revision 22
# speedup vs baseline: 18480.5634x; 16430.2042x over previous
"""CrossAttentionBlock kernel for Trainium2 (Bass/Tile), 8-core data-parallel.

Strategy:
  - One batch element per NeuronCore (B=8 -> 8 cores), no collectives.
  - All activations kept feature-major ("transposed", [feature, token]) on
    device so every matmul contraction lands on the partition axis.
    Host pre-transposes query/key_value per core and pre-tiles all weight
    matrices into [m_tile, p, k_tile, col] blocks so every DMA is contiguous.
  - Matmuls run in float32r (TF32-like, 1 cycle/row) with fp32 PSUM accum.
  - LayerNorm reductions (over features = partitions) use ones-matmuls on the
    PE with M=128 so the stats come out pre-replicated across partitions.
  - Softmax: scores computed [key, query]-major; padding mask and 1/sqrt(d)
    scale fold into the Exp activation (bias/scale); the softmax denominator
    comes free as an extra ones-column in the attn@V matmul; no max-
    subtraction is needed (scores are small by construction).
  - attn_weights output (mean over heads of softmax) uses a second scores
    pass in [query, key] orientation where exp(s/8 - ln(16*l)) normalizes
    and averages for free; head-accumulation runs on GPSIMD to keep the
    vector engine off the critical path.
"""

import time

import ml_dtypes
import numpy as np

import concourse.bass as bass
import concourse.tile as tile
from concourse import bacc, mybir
from concourse.bass_utils import run_bass_kernel_spmd

AF = mybir.ActivationFunctionType
ALU = mybir.AluOpType

f32 = mybir.dt.float32
f32r = mybir.dt.float32r
bf16 = mybir.dt.bfloat16

P = 128
D = 1024
H = 16
HD = 64
FF = 4096
B = 8
NQ = 512
NKV = 1024
KT = D // P  # 8 k-tiles over D
FT = FF // P  # 32 tiles over FF
MASK_NEG = -50.0
EPS = 1e-5

LAST_RESULTS = None


def _ln_partition_major(nc, work, psum_st, eps_col, zero_col, x_sb, out_sb, free_len,
                        ones_sb, g_pm, b_pm):
    """LayerNorm over the partition (feature) axis of x_sb [P, KT, free_len].

    Stats are computed with ones-matmuls (M=128 -> replicated across
    partitions).  g_pm/b_pm are [P, KT] per-partition gamma/beta columns.
    out_sb may alias x_sb (in-place).
    """
    n_chunks = free_len // 512
    for c in range(n_chunks):
        cs = slice(c * 512, (c + 1) * 512)
        ps_mu = psum_st.tile([P, 512], f32, tag="mm", name="ps_mu")
        ps_sq = psum_st.tile([P, 512], f32, tag="mm", name="ps_sq")
        sq_tiles = []
        for kt in range(KT):
            sq = work.tile([P, 512], f32r, tag="ln_sq", bufs=2, name="ln_sq")
            if kt % 2 == 1:
                nc.scalar.activation(sq[:], x_sb[:, kt, cs], AF.Square,
                                     bias=zero_col[:], scale=1.0)
            elif kt in (2, 6):
                nc.gpsimd.tensor_mul(sq[:], x_sb[:, kt, cs], x_sb[:, kt, cs])
            else:
                nc.vector.tensor_mul(sq[:], x_sb[:, kt, cs], x_sb[:, kt, cs])
            sq_tiles.append(sq)
            nc.tensor.matmul(
                ps_sq[:], ones_sb[:], sq[:], start=kt == 0, stop=kt == KT - 1
            )
        for kt in range(KT):
            nc.tensor.matmul(
                ps_mu[:], ones_sb[:], x_sb[:, kt, cs], start=kt == 0, stop=kt == KT - 1
            )
        mu = work.tile([P, 512], f32, tag="ln_mu", bufs=1, name="ln_mu")
        nc.vector.tensor_scalar_mul(mu[:], ps_mu[:], 1.0 / D)
        musq = work.tile([P, 512], f32, tag="ln_musq", bufs=1, name="ln_musq")
        nc.scalar.activation(musq[:], mu[:], AF.Square, bias=zero_col[:])
        var = work.tile([P, 512], f32, tag="ln_var", bufs=1, name="ln_var")
        nc.vector.scalar_tensor_tensor(
            out=var[:],
            in0=ps_sq[:],
            scalar=1.0 / D,
            in1=musq[:],
            op0=ALU.mult,
            op1=ALU.subtract,
        )
        std = work.tile([P, 512], f32, tag="ln_std", bufs=1, name="ln_std")
        nc.scalar.activation(std[:], var[:], AF.Sqrt, bias=eps_col[:])
        rstd = work.tile([P, 512], f32, tag="ln_rstd", bufs=1, name="ln_rstd")
        nc.vector.reciprocal(rstd[:], std[:])
        for kt in range(KT):
            xc = work.tile([P, 512], f32, tag="ln_xc", bufs=3, name="ln_xc")
            eng = nc.gpsimd if kt % 3 == 1 else nc.vector
            eng.tensor_sub(xc[:], x_sb[:, kt, cs], mu[:])
            eng.tensor_mul(xc[:], xc[:], rstd[:])
            # gamma * xc + beta on the scalar engine
            nc.scalar.activation(
                out_sb[:, kt, cs], xc[:], AF.Identity,
                bias=b_pm[:, kt : kt + 1], scale=g_pm[:, kt : kt + 1],
            )


def build_nc():
    nc = bacc.Bacc("TRN2", target_bir_lowering=False, debug=False)

    # ---- DRAM I/O ----
    q_t = nc.dram_tensor("query_t", [D, NQ], f32r, kind="ExternalInput")
    kv_t = nc.dram_tensor("kv_t", [D, NKV], f32r, kind="ExternalInput")
    maskb_d = nc.dram_tensor("maskbias_pm", [P, KT], f32, kind="ExternalInput")
    maskbit_d = nc.dram_tensor("maskbit16_pm", [P, KT], f32, kind="ExternalInput")
    w_qk_d = nc.dram_tensor("w_qk", [16, P, KT, P], f32r, kind="ExternalInput")
    w_v_d = nc.dram_tensor("w_v", [P, KT, D], f32r, kind="ExternalInput")
    ipb_d = nc.dram_tensor("ipb_pm", [P, 16], f32, kind="ExternalInput")
    bv_d = nc.dram_tensor("bv_rep", [P, D], f32, kind="ExternalInput")
    w_out_d = nc.dram_tensor("w_out", [KT, P, KT, P], f32r, kind="ExternalInput")
    outb_d = nc.dram_tensor("outb_pm", [P, KT], f32, kind="ExternalInput")
    gb_d = nc.dram_tensor("gb_pm", [P, 6, KT], f32, kind="ExternalInput")
    w_ff1_d = nc.dram_tensor("w_ff1", [FT, P, KT, P], bf16, kind="ExternalInput")
    ff1b_d = nc.dram_tensor("ff1b_pm", [P, FT], f32, kind="ExternalInput")
    w_ff2_d = nc.dram_tensor("w_ff2", [KT, P, FT, P], bf16, kind="ExternalInput")
    ff2b_d = nc.dram_tensor("ff2b_pm", [P, KT], f32, kind="ExternalInput")
    ones_d = nc.dram_tensor("ones_in", [P, P], f32r, kind="ExternalInput")

    x_t_out = nc.dram_tensor("x_t_out", [D, NQ], f32, kind="ExternalOutput")
    attn_out = nc.dram_tensor("attn_t_out", [NKV, NQ], f32, kind="ExternalOutput")

    with tile.TileContext(nc) as tc:
        # ---------- long-lived pools ----------
        # LEFT stack: const, p_x, p_qorig, p_ctx, p_qkT, p_v, p_att (LIFO)
        # RIGHT stack: p_kv, p_win, p_qln, work_in | p_wv | p_wmid, work_out
        const = tc.alloc_tile_pool(name="const", bufs=1, side="left")
        psum_mm = tc.alloc_tile_pool(name="psum_mm", bufs=8, space="PSUM")

        ones_sb = const.tile([P, P], f32r, tag="ones", name="ones_sb")
        nc.sync.dma_start(ones_sb[:], ones_d[:])
        eps_col = const.tile([P, 1], f32, tag="eps", name="eps_col")
        nc.vector.memset(eps_col[:], EPS)
        zero_col = const.tile([P, 1], f32, tag="zero", name="zero_col")
        nc.vector.memset(zero_col[:], 0.0)
        maskb = const.tile([P, KT], f32, tag="maskb", name="maskb")
        nc.sync.dma_start(maskb[:], maskb_d[:])
        maskbit = const.tile([P, KT], f32, tag="maskbit", name="maskbit")
        nc.sync.dma_start(maskbit[:], maskbit_d[:])
        ipb = const.tile([P, 16], f32, tag="ipb", name="ipb")
        nc.sync.dma_start(ipb[:], ipb_d[:])
        bv = const.tile([P, D], f32, tag="bv", name="bv")
        nc.sync.dma_start(bv[:], bv_d[:])
        outb = const.tile([P, KT], f32, tag="outb", name="outb")
        nc.sync.dma_start(outb[:], outb_d[:])
        gb = const.tile([P, 6, KT], f32, tag="gb", name="gb")
        nc.sync.dma_start(gb[:], gb_d[:])
        ff1b = const.tile([P, FT], f32, tag="ff1b", name="ff1b")
        nc.sync.dma_start(ff1b[:], ff1b_d[:])
        ff2b = const.tile([P, KT], f32, tag="ff2b", name="ff2b")
        nc.sync.dma_start(ff2b[:], ff2b_d[:])

        # ---------- phase pools ----------
        p_x = tc.alloc_tile_pool(name="p_x", bufs=1, side="left")
        p_qorig = tc.alloc_tile_pool(name="p_qorig", bufs=1, side="left")
        p_qkT = tc.alloc_tile_pool(name="p_qkT", bufs=1, side="left")
        p_kv = tc.alloc_tile_pool(name="p_kv", bufs=1, side="right")
        p_wv = tc.alloc_tile_pool(name="p_wv", bufs=2, side="right")
        p_win = tc.alloc_tile_pool(name="p_win", bufs=2, side="right")
        p_qln = tc.alloc_tile_pool(name="p_qln", bufs=1, side="right")
        work_in = tc.alloc_tile_pool(name="work_in", bufs=1, side="right")

        # ---- load activations (feature-major) ----
        q_orig = p_qorig.tile([P, KT, NQ], f32r, tag="q_orig", name="q_orig")
        for t in range(KT):
            nc.sync.dma_start(q_orig[:, t, :], q_t[t * P : (t + 1) * P, :])
        kv_sb = p_kv.tile([P, KT, NKV], f32r, tag="kv", name="kv_sb")
        for t in range(KT):
            nc.sync.dma_start(kv_sb[:, t, :], kv_t[t * P : (t + 1) * P, :])

        # ---- input layernorms (kv in-place) ----
        qln = p_qln.tile([P, KT, NQ], f32r, tag="qln", name="qln")
        _ln_partition_major(
            nc, work_in, psum_mm, eps_col, zero_col, q_orig, qln, NQ, ones_sb,
            gb[:, 0, :], gb[:, 1, :],
        )
        _ln_partition_major(
            nc, work_in, psum_mm, eps_col, zero_col, kv_sb, kv_sb, NKV, ones_sb,
            gb[:, 2, :], gb[:, 3, :],
        )

        # ---- in-projection: q.T, k.T (feature-major) ----
        qT = p_qkT.tile([P, KT, NQ], f32r, tag="qT", name="qT")
        kT = p_qkT.tile([P, KT, NKV], f32r, tag="kT", name="kT")
        wv_chunks = []
        for c in range(2):
            wv_c = p_wv.tile([P, KT, 512], f32r, tag="wv", bufs=2, name="wv_c")
            nc.sync.dma_start(wv_c[:], w_v_d[:, :, c * 512 : (c + 1) * 512])
            wv_chunks.append(wv_c)
        for m in range(16):
            wt = p_win.tile([P, KT, P], f32r, tag="w", name="w_in")
            nc.sync.dma_start(wt[:], w_qk_d[m])
            if m < 8:  # q: one 512-wide chunk
                ps = psum_mm.tile([P, 512], f32, tag="mm", name="ps_q")
                for kt in range(KT):
                    nc.tensor.matmul(
                        ps[:], wt[:, kt, :], qln[:, kt, :],
                        start=kt == 0, stop=kt == KT - 1,
                    )
                if m % 2 == 0:
                    nc.scalar.activation(
                        qT[:, m, :], ps[:], AF.Identity, bias=ipb[:, m : m + 1]
                    )
                else:
                    nc.vector.tensor_scalar_add(
                        out=qT[:, m, :], in0=ps[:], scalar1=ipb[:, m : m + 1]
                    )
            else:  # k: two 512-wide chunks
                for c in range(2):
                    cs = slice(c * 512, (c + 1) * 512)
                    ps = psum_mm.tile([P, 512], f32, tag="mm", name="ps_k")
                    for kt in range(KT):
                        nc.tensor.matmul(
                            ps[:], wt[:, kt, :], kv_sb[:, kt, cs],
                            start=kt == 0, stop=kt == KT - 1,
                        )
                    if m % 2 == 0:
                        nc.scalar.activation(
                            kT[:, m - 8, cs], ps[:], AF.Identity,
                            bias=ipb[:, m : m + 1],
                        )
                    else:
                        nc.vector.tensor_scalar_add(
                            out=kT[:, m - 8, cs], in0=ps[:],
                            scalar1=ipb[:, m : m + 1],
                        )
        work_in.release()
        p_qln.release()
        p_win.release()

        # ---- in-projection: v (token-major, [v | one] per head) ----
        p_v = tc.alloc_tile_pool(name="p_v", bufs=1, side="left")
        v_sb = p_v.tile([P, KT, H, HD + 1], bf16, tag="v", name="v_sb")
        for tt in range(KT):
            nc.vector.tensor_copy(v_sb[:, tt, :, HD : HD + 1], ones_sb[:, 0:H, None])
        for c in range(2):
            wv_c = wv_chunks[c]
            for tt in range(KT):
                ps = psum_mm.tile([P, 512], f32, tag="mm", name="ps_v")
                for kt in range(KT):
                    nc.tensor.matmul(
                        ps[:],
                        kv_sb[:, kt, tt * P : (tt + 1) * P],
                        wv_c[:, kt, :],
                        start=kt == 0,
                        stop=kt == KT - 1,
                    )
                nc.vector.tensor_add(
                    v_sb[:, tt, 8 * c : 8 * c + 8, 0:HD],
                    ps[:].rearrange("p (j d) -> p j d", d=HD),
                    bv[:, c * 512 : (c + 1) * 512].rearrange(
                        "p (j d) -> p j d", d=HD
                    ),
                )
        p_wv.release()
        p_kv.release()

        # ---- attention ----
        p_ctx = tc.alloc_tile_pool(name="p_ctx", bufs=1, side="right")
        p_att = tc.alloc_tile_pool(name="p_att", bufs=1, side="left")
        ctx_sb = p_ctx.tile([P, KT, NQ], f32r, tag="ctx", name="ctx_sb")
        attn_acc = p_att.tile([P, KT, NQ], f32, tag="attn_acc", name="attn_acc")
        nc.vector.memset(attn_acc[:], 0.0)
        for h in [x for ht_ in range(KT) for x in (2 * ht_ + 1, 2 * ht_)]:
            hb = (h % 2) * 64
            ht = h // 2
            hs = slice(hb, hb + 64)
            # scores pass 1: s.T [key, query]; exp with mask+scale folded
            p_sb = p_att.tile([P, KT, NQ], bf16, tag="p", bufs=2, name="p_sb")
            for tkt in range(KT):
                ps = psum_mm.tile([P, 512], f32, tag="mm", name="ps_s1")
                nc.tensor.matmul(
                    ps[:],
                    kT[hs, ht, tkt * P : (tkt + 1) * P],
                    qT[hs, ht, :],
                    start=True,
                    stop=True,
                )
                nc.scalar.activation(
                    p_sb[:, tkt, :], ps[:], AF.Exp,
                    bias=maskb[:, tkt : tkt + 1], scale=0.125,
                )
            # ctx.T + softmax denominator (ones column)
            ctx_ps = psum_mm.tile([P, 512], f32, tag="mm", name="ps_ctx")
            for tt in range(KT):
                nc.tensor.matmul(
                    ctx_ps[0:65, :],
                    v_sb[:, tt, h, :],
                    p_sb[:, tt, :],
                    start=tt == 0,
                    stop=tt == KT - 1,
                )
            # broadcast l across partitions with a K=1 ones-matmul, then
            # reciprocal -> r_rep [P, 512]
            l_row = p_att.tile([P, 512], f32r, tag="lrow", bufs=2, name="l_row")
            nc.scalar.activation(l_row[64:65, :], ctx_ps[64:65, :], AF.Identity,
                                 bias=zero_col[64:65, :])
            l_rep = psum_mm.tile([P, 512], f32, tag="mm", name="l_rep")
            nc.tensor.matmul(
                l_rep[:], ones_sb[64:65, :], l_row[64:65, :], start=True, stop=True
            )
            r_rep = p_att.tile([P, 512], f32, tag="rrep", bufs=2, name="r_rep")
            nc.vector.reciprocal(r_rep[:], l_rep[:])
            # normalized ctx into feature-major ctx_sb
            if h % 2 == 0:
                nc.vector.tensor_mul(
                    ctx_sb[0:64, ht, :], ctx_ps[0:64, :], r_rep[0:64, :]
                )
            else:
                ctmp = p_att.tile([64, 512], f32r, tag="ctmp", bufs=1, name="ctmp")
                nc.vector.tensor_mul(ctmp[:], ctx_ps[0:64, :], r_rep[0:64, :])
                nc.sync.dma_start(ctx_sb[64:128, ht, :], ctmp[:])
            # attn accumulation in [key, query] orientation:
            # acc[tk, tq] += p[tk, tq] * r[tq]   (mean/mask applied at the end)
            # bf16 pairs hit the DVE 2x mode; adds split between Pool and DVE
            r_bf = p_att.tile([P, 512], bf16, tag="rbf", bufs=2, name="r_bf")
            nc.vector.tensor_copy(r_bf[:], r_rep[:])
            for tkp in range(KT // 2):
                pr = p_att.tile([P, 2, 512], bf16, tag="pr", bufs=3, name="pr")
                nc.vector.tensor_mul(
                    pr[:],
                    p_sb[:, 2 * tkp : 2 * tkp + 2, :],
                    r_bf[:, None, :].to_broadcast([P, 2, 512]),
                )
                eng = nc.gpsimd if tkp < 3 else nc.vector
                eng.tensor_add(
                    attn_acc[:, 2 * tkp : 2 * tkp + 2, :],
                    attn_acc[:, 2 * tkp : 2 * tkp + 2, :],
                    pr[:],
                )

        # mean over heads + zero out masked keys, then store (transposed)
        for tkt in range(KT):
            nc.vector.tensor_scalar_mul(
                out=attn_acc[:, tkt, :], in0=attn_acc[:, tkt, :],
                scalar1=maskbit[:, tkt : tkt + 1],
            )
            nc.sync.dma_start(
                attn_out[tkt * P : (tkt + 1) * P, :], attn_acc[:, tkt, :]
            )
        p_att.release()
        p_v.release()
        p_qkT.release()

        # ---- out-projection + residual ----
        p_wmid = tc.alloc_tile_pool(name="p_wmid", bufs=3, side="right")
        work_out = tc.alloc_tile_pool(name="work_out", bufs=1, side="right")
        x_sb = p_x.tile([P, KT, NQ], f32r, tag="x", name="x_sb")
        for m in range(KT):
            wt = p_wmid.tile([P, KT, P], f32r, tag="w", name="w_out_t")
            nc.sync.dma_start(wt[:], w_out_d[m])
            ps = psum_mm.tile([P, 512], f32, tag="mm", name="ps_o")
            for kt in range(KT):
                nc.tensor.matmul(
                    ps[:], wt[:, kt, :], ctx_sb[:, kt, :],
                    start=kt == 0, stop=kt == KT - 1,
                )
            # x = (attended + out_b) + query
            nc.vector.scalar_tensor_tensor(
                out=x_sb[:, m, :],
                in0=ps[:],
                scalar=outb[:, m : m + 1],
                in1=q_orig[:, m, :],
                op0=ALU.add,
                op1=ALU.add,
            )
        p_qorig.release()

        # ---- FFN ----
        p_ffn = tc.alloc_tile_pool(name="p_ffn", bufs=1, side="left")
        xln = p_ffn.tile([P, KT, NQ], bf16, tag="xln", name="xln")
        _ln_partition_major(
            nc, work_out, psum_mm, eps_col, zero_col, x_sb, xln, NQ, ones_sb,
            gb[:, 4, :], gb[:, 5, :],
        )
        h_sb = p_ffn.tile([P, FT, NQ], bf16, tag="h", name="h_sb")
        for m in range(FT):
            wt = p_wmid.tile([P, KT, P], bf16, tag="wb", name="w_ff1_t")
            nc.sync.dma_start(wt[:], w_ff1_d[m])
            ps = psum_mm.tile([P, 512], f32, tag="mm", name="ps_f1")
            for kt in range(KT):
                nc.tensor.matmul(
                    ps[:], wt[:, kt, :], xln[:, kt, :],
                    start=kt == 0, stop=kt == KT - 1,
                )
            nc.scalar.activation(
                h_sb[:, m, :], ps[:], AF.Gelu, bias=ff1b[:, m : m + 1]
            )
        out_sb = p_ffn.tile([P, KT, NQ], f32, tag="out", name="out_sb")
        for m in range(KT):
            wt = p_wmid.tile([P, FT, P], bf16, tag="wff2", bufs=2, name="w_ff2_t")
            nc.sync.dma_start(wt[:], w_ff2_d[m])
            ps = psum_mm.tile([P, 512], f32, tag="mm", name="ps_f2")
            for kt in range(FT):
                nc.tensor.matmul(
                    ps[:], wt[:, kt, :], h_sb[:, kt, :],
                    start=kt == 0, stop=kt == FT - 1,
                )
            nc.vector.scalar_tensor_tensor(
                out=out_sb[:, m, :],
                in0=ps[:],
                scalar=ff2b[:, m : m + 1],
                in1=x_sb[:, m, :],
                op0=ALU.add,
                op1=ALU.add,
            )
            nc.sync.dma_start(x_t_out[m * P : (m + 1) * P, :], out_sb[:, m, :])

        p_ffn.release()
        p_x.release()
        work_out.release()
        p_wmid.release()
        p_ctx.release()
        const.release()
        psum_mm.release()

    nc.compile()
    return nc


_NC_CACHE = None


def _get_nc():
    global _NC_CACHE
    if _NC_CACHE is None:
        _NC_CACHE = build_nc()
    return _NC_CACHE


def _prep_shared(in_proj_w, in_proj_b, out_w, out_b, nq_gamma, nq_beta, nkv_gamma,
                 nkv_beta, nff_gamma, nff_beta, ff1_w, ff1_b, ff2_w, ff2_b):
    def pm(v, nt):  # per-partition layout [P, nt]
        return np.ascontiguousarray(np.asarray(v, np.float32).reshape(nt, P).T)

    def wtiles(w_t, mt):  # [m, p, kt, c] tiled layout from [in, out] matrix
        kt = w_t.shape[0] // P
        return np.ascontiguousarray(w_t.reshape(kt, P, mt, P).transpose(2, 1, 0, 3))

    ipw_t = np.asarray(in_proj_w, np.float32).T  # (1024, 3072)
    return {
        "w_qk": wtiles(np.ascontiguousarray(ipw_t[:, : 2 * D]), 16),
        "w_v": np.ascontiguousarray(
            ipw_t[:, 2 * D :].reshape(KT, P, D).transpose(1, 0, 2)
        ),
        "ipb_pm": pm(np.asarray(in_proj_b, np.float32)[: 2 * D], 16),
        "bv_rep": np.ascontiguousarray(
            np.broadcast_to(np.asarray(in_proj_b, np.float32)[2 * D :], (P, D))
        ),
        "w_out": wtiles(np.asarray(out_w, np.float32).T, KT),
        "outb_pm": pm(out_b, KT),
        "gb_pm": np.ascontiguousarray(
            np.stack(
                [pm(v, KT) for v in
                 [nq_gamma, nq_beta, nkv_gamma, nkv_beta, nff_gamma, nff_beta]],
                axis=1,
            )
        ),
        "w_ff1": wtiles(np.asarray(ff1_w, np.float32).T, FT).astype(
            ml_dtypes.bfloat16
        ),
        "ff1b_pm": pm(ff1_b, FT),
        "w_ff2": wtiles(np.asarray(ff2_w, np.float32).T, KT).astype(
            ml_dtypes.bfloat16
        ),
        "ff2b_pm": pm(ff2_b, KT),
    }


def kernel(query, key_value, key_padding_mask, nq_gamma, nq_beta, nkv_gamma,
           nkv_beta, in_proj_w, in_proj_b, out_w, out_b, nff_gamma, nff_beta,
           ff1_w, ff1_b, ff2_w, ff2_b):
    global LAST_RESULTS
    query = np.asarray(query, np.float32)
    key_value = np.asarray(key_value, np.float32)
    mask = np.asarray(key_padding_mask)

    shared = _prep_shared(in_proj_w, in_proj_b, out_w, out_b, nq_gamma, nq_beta,
                          nkv_gamma, nkv_beta, nff_gamma, nff_beta, ff1_w,
                          ff1_b, ff2_w, ff2_b)

    in_maps = []
    for b in range(B):
        mb = np.where(mask[b], np.float32(MASK_NEG), np.float32(0.0)).astype(
            np.float32
        )
        mbit = np.where(mask[b], np.float32(0.0), np.float32(1.0 / 16.0)).astype(
            np.float32
        )
        m = dict(shared)
        m["query_t"] = np.ascontiguousarray(query[b].T)
        m["kv_t"] = np.ascontiguousarray(key_value[b].T)
        m["maskbias_pm"] = np.ascontiguousarray(mb.reshape(KT, P).T)
        m["ones_in"] = np.ones((P, P), np.float32)
        m["maskbit16_pm"] = np.ascontiguousarray(mbit.reshape(KT, P).T)
        in_maps.append(m)

    nc = _get_nc()
    t0 = time.monotonic()
    res = run_bass_kernel_spmd(nc, in_maps, core_ids=list(range(B)))
    t1 = time.monotonic()
    LAST_RESULTS = {"res": res, "wall_s": t1 - t0}

    x = np.stack([res.results[b]["x_t_out"].T for b in range(B)])
    attn = np.stack([res.results[b]["attn_t_out"].T for b in range(B)])
    return (np.ascontiguousarray(x), np.ascontiguousarray(attn))
